# revision 27
# baseline (speedup 1.0000x reference)
"""Causal single-head attention (B=4, S=4096, D=512, dk=64) on 8 Trainium2
NeuronCores via Bass/Tile.

Sharding: core c handles batch b = c//2, query parity p = c%2 — the four
512-row query chunks with global chunk index 2j+p, j=0..3.  Work per job j
is uniform across cores (E[j] = 8j+8 key-tiles of 128); causal boundary
differences between parities are handled by per-core 0/1 mask tensors
(data, not program), so a single SPMD program serves all 8 cores.

Per-core pipeline (emission interleaves projection chunks with attention
jobs so the in-order PE stream stays dense and the HAM clock-gate keeps
the PE at 2.4 GHz):

  group j:  project qT chunk j;  project kT/vT for x2 chunks 2j, 2j+1;
            PE-transpose vT tiles into v_aug [128, 65] tiles (column 64
            is 1.0 so the PV matmul also accumulates the softmax
            denominator);  then run attention job j over key tiles
            t < E[j], two tiles per scores-PSUM buffer:
               scT(t,t+1) = kT_tile^T qT_j          (PE -> PSUM pair)
               attnT = exp(scT / 8)                 (ACT; no max needed,
                                                     scores in [0, ~6])
               diagonal-band tiles multiply by a mask tile (DVE)
               outT[j] += v_aug(t)^T attnT(t)       (PE, deferred one
                                                     pair so PE never
                                                     waits on exp)
            finalize: PE-transpose outT, divide by denominator column
            (DVE reciprocal + per-partition multiply), DMA out.

Matmul operands are bf16 (PE runs 1 cycle/row; fp32 is 4 and float32r is
SBUF-bandwidth-capped on this toolchain); accumulation is fp32 in PSUM.
"""
import os
import numpy as np
import ml_dtypes

import bass_rust
import concourse.bass as bass
import concourse.tile as tile
from concourse import mybir
from concourse.bass_utils import run_bass_kernel_spmd
from concourse.masks import make_identity

# ---------------------------------------------------------------- constants
P = 128          # partitions / sk tile
D = 512          # model dim
DK = 64          # key dim
S = 4096         # sequence
B = 4            # batch
CH = 512         # sq chunk width (one job)
NJ = 4           # jobs per core
KD = D // P      # k-tiles in the D contraction
NSK = S // P     # sk tiles
SQ = NJ * CH     # q rows per core
N_CORES = 8

F32 = mybir.dt.float32
BF16 = mybir.dt.bfloat16
F32R = mybir.dt.float32r

_DTMAP = {"bf16": BF16, "f32r": F32R, "f32": F32}
_NPMAP = {"bf16": ml_dtypes.bfloat16, "f32r": np.float32, "f32": np.float32}

_CFG = {
    "dt_proj": os.environ.get("K_DT_PROJ", "bf16"),
    "dt_sc": os.environ.get("K_DT_SC", "bf16"),
    "dt_pv": os.environ.get("K_DT_PV", "bf16"),
    "relu_dve": os.environ.get("K_RELU_DVE", "1") == "1",
    "mask_pool": os.environ.get("K_MASK_POOL", "0") == "1",
    "trace": os.environ.get("K_TRACE", "0") == "1",
}


# ------------------------------------------------- walrus codegen workarounds
def _patch_tile_drain():
    """This neuronxcc rejects >1 sync wait on a CTRL (Drain) instruction;
    TileContext's tail drain carries one wait per live semaphore.  Split the
    waits onto dedicated SP nops, one wait each."""
    from concourse.tile import TileContext

    if getattr(TileContext, "_drain_patched", False):
        return

    def _patched(self, tick_clock, wait_clock):
        nc = self.nc
        probe = nc.sync.nop(nofuse=True, hint="tail_wait_probe")
        wait_clock.add_sem_waits(
            probe.ins, bass_rust.ScopedClock({None: tick_clock.global_clock})
        )
        si = probe.ins.sync_info
        waits = list(si.on_wait) if si is not None else []
        probe.ins.sync_info = bass_rust.SyncInfo(on_wait=waits[:1], on_update=[])
        for w in waits[1:]:
            carrier = nc.sync.nop(nofuse=True, hint="tail_wait")
            carrier.ins.sync_info = bass_rust.SyncInfo(on_wait=[w], on_update=[])
        nc.sync.drain()

        nc.all_engine_barrier()
        assert self.sems is not None
        popped = nc._tile_sem_poison_stack.pop()
        assert popped is self._sem_poison
        nc.clear_and_free_semaphores(list(self.sems.allocated().values()))
        nc.all_engine_barrier()

    TileContext._drain_and_barrier = _patched
    TileContext._drain_patched = True


def _split_sync_waits(nc, max_waits: int = 1):
    """walrus here rejects >1 sync wait on at least CTRL and S3_LW (weight
    load) instruction structs.  Hoist excess waits onto same-engine NOPs
    placed immediately before the instruction (engine streams execute block
    order, so the waits still gate the instruction)."""
    counter = [0]
    for fn in nc.m.functions:
        for bb in fn.blocks:
            changed = False
            new = []
            for inst in bb.instructions:
                si = inst.sync_info
                waits = list(si.on_wait) if si is not None else []
                if len(waits) > max_waits:
                    changed = True
                    for w in waits[:-max_waits]:
                        counter[0] += 1
                        nop = bass_rust.InstNoOp(
                            name=f"I-waitsplit-{counter[0]}", engine=inst.engine
                        )
                        nop.bass_nofuse = True
                        nop.sync_info = bass_rust.SyncInfo(
                            on_wait=[w], on_update=[]
                        )
                        new.append(nop)
                    inst.sync_info = bass_rust.SyncInfo(
                        on_wait=waits[-max_waits:], on_update=list(si.on_update)
                    )
                new.append(inst)
            if changed:
                bb.instructions = new


# ---------------------------------------------------------------- program
def _build_program(causal: bool):
    _patch_tile_drain()
    nc = bass.Bass()

    DT_X = _DTMAP[_CFG["dt_proj"]]
    DT_QK = _DTMAP[_CFG["dt_sc"]]
    DT_AT = _DTMAP[_CFG["dt_pv"]]

    # chunk-contiguous host layouts: one DMA per 512-column chunk, each a
    # fully contiguous [128, KD*CH] block
    x1c = nc.declare_dram_parameter("x1c", [SQ // CH, P, KD * CH], DT_X,
                                    isOutput=False)
    x2c = nc.declare_dram_parameter("x2c", [S // CH, P, KD * CH], DT_X,
                                    isOutput=False)
    # packed projection weights: [Wq|Wq|Wk|Wk|Wv] (q/k duplicated so the
    # projection emits qT/kT replicated across both partition halves, which
    # lets the two K=64 score matmuls of a pair run concurrently on disjoint
    # PE row groups)
    WM = 5 * DK
    wall = nc.declare_dram_parameter("wall", [P, KD * WM], DT_X, isOutput=False)
    ball = nc.declare_dram_parameter("ball", [P, 3], F32, isOutput=False)
    masks = nc.declare_dram_parameter("masks", [8, P, CH], DT_AT, isOutput=False)
    ones = nc.declare_dram_parameter("ones", [P, NSK], DT_AT, isOutput=False)
    out = nc.declare_dram_parameter("out", [SQ, DK], F32, isOutput=True)

    E = [8 * j + 8 for j in range(NJ)] if causal else [NSK] * NJ

    Exp = mybir.ActivationFunctionType.Exp
    Relu = mybir.ActivationFunctionType.Relu

    def bias_relu(dst, src_psum, bias_sb):
        """dst = relu(src + bias), bias per-partition [p,1]."""
        if _CFG["relu_dve"]:
            nc.vector.tensor_scalar(
                dst, src_psum, bias_sb, 0.0,
                mybir.AluOpType.add, mybir.AluOpType.max,
            )
        else:
            nc.scalar.activation(out=dst, in_=src_psum, func=Relu,
                                 bias=bias_sb, scale=1.0)

    with tile.TileContext(nc) as tc:
        with (
            tc.tile_pool(name="const", bufs=1) as const,
            tc.tile_pool(name="xin", bufs=8) as xin,
            tc.tile_pool(name="resident", bufs=1) as res,
            tc.tile_pool(name="attn", bufs=6) as attn,
            tc.tile_pool(name="ostage", bufs=4) as ostage,
            tc.tile_pool(name="outps", bufs=2, space="PSUM") as outps,
            tc.tile_pool(name="pps", bufs=2, space="PSUM") as pps,
            tc.tile_pool(name="sps", bufs=2, space="PSUM") as sps,
        ):
            # ---------------- constants
            w_sb = const.tile([P, KD, WM], DT_X)
            b_sb = const.tile([P, 3], F32)
            ident = const.tile([P, P], F32)
            make_identity(nc, ident)
            identv = const.tile([P, P], DT_X)
            make_identity(nc, identv)

            qT_sb = res.tile([P, SQ], DT_QK)
            kT_sb = res.tile([P, S], DT_QK)
            vT_sb = res.tile([DK, S], DT_X)
            # inner stride 80 keeps each [*, st, 0:64] slice 32B-aligned for
            # the SBUF->SBUF DMA transpose that fills it
            VP = 80
            v_sb = res.tile([P, NSK, VP], DT_AT)
            nc.sync.dma_start(
                out=v_sb[:, :, DK:DK + 1],
                in_=ones.rearrange("p (n o) -> p n o", o=1),
            )

            # ---- all input DMAs issued up-front (SP executes triggers in
            # program order; interleaving them with output DMAs would stall
            # the input stream behind compute-dependent stores).  The first
            # x1/x2 chunks are split per-kd so the first projection matmuls
            # start as soon as their own 128KB slice lands.
            x1_first = [
                xin.tile([P, CH], DT_X, name=f"x1f{k}", tag="x1f")
                for k in range(KD)
            ]
            x2_first = [
                xin.tile([P, CH], DT_X, name=f"x2f{k}", tag="x2f")
                for k in range(KD)
            ]
            x1_tiles = [None] + [
                xin.tile([P, KD, CH], DT_X, name=f"x1t{c}", tag="x1c")
                for c in range(1, SQ // CH)
            ]
            x2_tiles = [None] + [
                xin.tile([P, KD, CH], DT_X, name=f"x2t{c}", tag="x2c")
                for c in range(1, S // CH)
            ]
            nc.sync.dma_start(out=w_sb, in_=wall.rearrange("p (kd m) -> p kd m", kd=KD))
            x1v0 = x1c[0].rearrange("p (kd s) -> p kd s", kd=KD)
            x2v0 = x2c[0].rearrange("p (kd s) -> p kd s", kd=KD)
            for k in range(KD):
                nc.sync.dma_start(out=x1_first[k], in_=x1v0[:, k, :])
            for k in range(KD):
                nc.sync.dma_start(out=x2_first[k], in_=x2v0[:, k, :])
            nc.sync.dma_start(out=b_sb, in_=ball[:, :])
            nc.sync.dma_start(
                out=x2_tiles[1],
                in_=x2c[1].rearrange("p (kd s) -> p kd s", kd=KD),
            )
            if causal:
                masks_sb = const.tile([P, 8, CH], DT_AT)
                nc.sync.dma_start(
                    out=masks_sb, in_=masks.rearrange("m p s -> p m s")
                )
            for ch in range(1, SQ // CH):
                nc.sync.dma_start(
                    out=x1_tiles[ch],
                    in_=x1c[ch].rearrange("p (kd s) -> p kd s", kd=KD),
                )
                for ch2 in (2 * ch, 2 * ch + 1):
                    nc.sync.dma_start(
                        out=x2_tiles[ch2],
                        in_=x2c[ch2].rearrange("p (kd s) -> p kd s", kd=KD),
                    )

            def proj_q_chunk(ch):
                pq = pps.tile([P, CH], F32, tag="pps")
                for kd in range(KD):
                    rhs = (x1_first[kd] if ch == 0
                           else x1_tiles[ch][:, kd, :])
                    nc.tensor.matmul(
                        pq, w_sb[:, kd, 0:P], rhs,
                        start=(kd == 0), stop=(kd == KD - 1),
                    )
                bias_relu(qT_sb[:, ch * CH:(ch + 1) * CH], pq, b_sb[:, 0:1])

            def proj_kv_chunk(ch):
                def rhs(kd):
                    return (x2_first[kd] if ch == 0
                            else x2_tiles[ch][:, kd, :])
                pk = pps.tile([P, CH], F32, tag="pps")
                for kd in range(KD):
                    nc.tensor.matmul(
                        pk, w_sb[:, kd, P:2 * P], rhs(kd),
                        start=(kd == 0), stop=(kd == KD - 1),
                    )
                bias_relu(kT_sb[:, ch * CH:(ch + 1) * CH], pk, b_sb[:, 1:2])
                pv = pps.tile([DK, CH], F32, tag="pps")
                for kd in range(KD):
                    nc.tensor.matmul(
                        pv, w_sb[:, kd, 2 * P:2 * P + DK], rhs(kd),
                        start=(kd == 0), stop=(kd == KD - 1),
                    )
                bias_relu(vT_sb[:, ch * CH:(ch + 1) * CH], pv, b_sb[0:DK, 2:3])

            def transpose_v(st):
                pt = pps.tile([P, DK], DT_X, tag="pps")
                nc.tensor.transpose(
                    pt, in_=vT_sb[:, st * P:(st + 1) * P],
                    identity=identv[:DK, :DK],
                )
                nc.vector.tensor_copy(v_sb[:, st, 0:DK], pt)

            def finalize_job(j, oT_ps):
                oT = ostage.tile([DK + 1, CH], F32, tag="oT")
                nc.vector.tensor_copy(oT, oT_ps)
                for blk in range(CH // P):
                    po = pps.tile([P, DK + 1], F32, tag="pps")
                    nc.tensor.transpose(
                        po,
                        in_=oT[:, blk * P:(blk + 1) * P],
                        identity=ident[:DK + 1, :DK + 1],
                    )
                    rec = ostage.tile([P, 1], F32, tag="rec")
                    nc.vector.reciprocal(rec, po[:, DK:DK + 1])
                    ot = ostage.tile([P, DK], F32, tag="ot")
                    nc.vector.tensor_scalar_mul(ot, po[:, 0:DK], rec)
                    r0 = j * CH + blk * P
                    nc.sync.dma_start(out=out[r0:r0 + P, :], in_=ot)

            def attention_job(j, new_tiles=(), finalize_prev=None):
                oT_ps = outps.tile([DK + 1, CH], F32, tag="outT")
                qslc = qT_sb[:, j * CH:(j + 1) * CH]
                npair = E[j] // 2
                DEPTH = 2        # PV trails the scores by 2 pairs so the PE
                pending = []     # stream never waits on a just-issued exp
                for pi in range(npair + DEPTH):
                    # spread the v transposes of this group's new key tiles
                    # across the early pairs (each tile is ready well before
                    # its PV consumes it)
                    for st in new_tiles[2 * pi:2 * pi + 2]:
                        transpose_v(st)
                    if pi == 1 and finalize_prev is not None:
                        finalize_prev()
                    if pi < npair:
                        sc = sps.tile([P, 1024], F32, tag="sc")
                        at = attn.tile([P, 1024], DT_AT, tag="attnT")
                        for half in range(2):
                            t = 2 * pi + half
                            lo = half * DK
                            nc.tensor.matmul(
                                sc[:, half * CH:(half + 1) * CH],
                                kT_sb[lo:lo + DK, t * P:(t + 1) * P],
                                qslc[lo:lo + DK, :],
                                start=True,
                                stop=True,
                            )
                        nc.scalar.activation(
                            out=at, in_=sc, func=Exp, scale=0.125
                        )
                        halves = []
                        for half in range(2):
                            t = 2 * pi + half
                            aslc = at[:, half * CH:(half + 1) * CH]
                            if causal and t >= E[j] - 8:
                                m = t - (E[j] - 8)
                                eng = (nc.gpsimd if _CFG["mask_pool"]
                                       else nc.vector)
                                eng.tensor_tensor(
                                    aslc, aslc, masks_sb[:, m, :],
                                    mybir.AluOpType.mult,
                                )
                            halves.append((t, aslc))
                        pending.append(halves)
                    if pi >= DEPTH:
                        for t, aslc in pending.pop(0):
                            nc.tensor.matmul(
                                oT_ps,
                                v_sb[:, t, 0:DK + 1],
                                aslc,
                                start=(t == 0),
                                stop=(t == E[j] - 1),
                                skip_group_check=True,
                            )
                return lambda: finalize_job(j, oT_ps)

            # ---------------- interleaved emission: group j feeds job j
            fin = None
            for j in range(NJ):
                proj_q_chunk(j)
                lo, hi = 2 * j, 2 * j + 2
                if not causal:
                    lo, hi = (0, S // CH) if j == 0 else (0, 0)
                new_tiles = []
                for ch in range(lo, hi):
                    proj_kv_chunk(ch)
                    new_tiles.extend(
                        ch * (CH // P) + blk for blk in range(CH // P)
                    )
                if not causal and j == 0:
                    # all keys needed up-front: transpose before the job
                    for st in new_tiles:
                        transpose_v(st)
                    new_tiles = []
                fin = attention_job(j, new_tiles, finalize_prev=fin)
            fin()

    _split_sync_waits(nc)
    return nc


_PROGRAMS = {}


def _program(causal: bool):
    if causal not in _PROGRAMS:
        _PROGRAMS[causal] = _build_program(causal)
    return _PROGRAMS[causal]


def _host_masks(parity: int) -> np.ndarray:
    """masks[m] multiplies the exp'd [sk=128, sq=512] tile of the job whose
    diagonal band covers key tiles [E-8, E); m = position in that band."""
    sk = np.arange(P)[:, None]
    sq = np.arange(CH)[None, :]
    m = np.zeros((8, P, CH), np.float32)
    for i in range(8):
        if parity == 1:
            if i < 4:
                m[i] = 1.0
            else:
                r = i - 4
                m[i] = (sq >= r * P + sk).astype(np.float32)
        else:
            if i < 4:
                m[i] = (sq >= i * P + sk).astype(np.float32)
            else:
                m[i] = 0.0
    return m


def _chunked(xt_rows: np.ndarray, np_x) -> np.ndarray:
    """[rows, D] -> [nch, 128, KD*CH] where [ch, p, kd*CH+s] =
    x[ch*CH+s, kd*128+p]."""
    nch = xt_rows.shape[0] // CH
    a = xt_rows.reshape(nch, CH, KD, P).transpose(0, 3, 2, 1)
    return np.ascontiguousarray(a.reshape(nch, P, KD * CH).astype(np_x))


def kernel(x1, x2, Wq, bq, Wk, bk, Wv, bv, apply_mask):
    np_x = _NPMAP[_CFG["dt_proj"]]
    np_at = _NPMAP[_CFG["dt_pv"]]
    x1 = np.asarray(x1, dtype=np.float32)
    x2 = np.asarray(x2, dtype=np.float32)
    Wq_f = np.asarray(Wq, np.float32)
    Wk_f = np.asarray(Wk, np.float32)
    Wv_f = np.asarray(Wv, np.float32)
    # packed [Wq|Wq|Wk|Wk|Wv] rearranged to the SBUF chunk layout
    Wcat = np.concatenate([Wq_f, Wq_f, Wk_f, Wk_f, Wv_f], axis=1)  # [D, 320]
    WM = Wcat.shape[1]
    wall_h = np.ascontiguousarray(
        Wcat.reshape(KD, P, WM).transpose(1, 0, 2).reshape(P, KD * WM)
    ).astype(np_x)
    ball_h = np.zeros((P, 3), np.float32)
    ball_h[:, 0] = np.concatenate([bq, bq])
    ball_h[:, 1] = np.concatenate([bk, bk])
    ball_h[0:DK, 2] = bv
    causal = bool(int(np.asarray(apply_mask)))

    nc = _program(causal)

    x2c_h = [_chunked(x2[b], np_x) for b in range(B)]
    ones_h = np.ones((P, NSK), np_at)
    masks_h = [_host_masks(p).astype(np_at) for p in range(2)]

    in_maps = []
    for core in range(N_CORES):
        b, p = core // 2, core % 2
        xb = x1[b]                                   # [S, D]
        rows = np.concatenate(
            [xb[(2 * j + p) * CH:(2 * j + p + 1) * CH] for j in range(NJ)], axis=0
        )                                            # [2048, D]
        in_maps.append({
            "x1c": _chunked(rows, np_x),
            "x2c": x2c_h[b],
            "wall": wall_h, "ball": ball_h,
            "masks": masks_h[p],
            "ones": ones_h,
        })

    res = run_bass_kernel_spmd(
        nc, in_maps, core_ids=list(range(N_CORES)), trace=_CFG["trace"]
    )
    kernel.last_result = res

    outp = np.empty((B, S, DK), np.float32)
    for core in range(N_CORES):
        b, p = core // 2, core % 2
        o = res.results[core]["out"]                 # [2048, 64]
        for j in range(NJ):
            outp[b, (2 * j + p) * CH:(2 * j + p + 1) * CH] = \
                o[j * CH:(j + 1) * CH]
    return outp


# revision 28
# speedup vs baseline: 1.0817x; 1.0817x over previous
"""Causal single-head attention (B=4, S=4096, D=512, dk=64) on 8 Trainium2
NeuronCores via Bass/Tile.

Sharding: core c handles batch b = c//2, query parity p = c%2 — the four
512-row query chunks with global chunk index 2j+p, j=0..3.  Work per job j
is uniform across cores (E[j] = 8j+8 key-tiles of 128); causal boundary
differences between parities are handled by per-core 0/1 mask tensors
(data, not program), so a single SPMD program serves all 8 cores.

Per-core pipeline (emission interleaves projection chunks with attention
jobs so the in-order PE stream stays dense and the HAM clock-gate keeps
the PE at 2.4 GHz):

  group j:  project qT chunk j;  project kT/vT for x2 chunks 2j, 2j+1;
            PE-transpose vT tiles into v_aug [128, 65] tiles (column 64
            is 1.0 so the PV matmul also accumulates the softmax
            denominator);  then run attention job j over key tiles
            t < E[j], two tiles per scores-PSUM buffer:
               scT(t,t+1) = kT_tile^T qT_j          (PE -> PSUM pair)
               attnT = exp(scT / 8)                 (ACT; no max needed,
                                                     scores in [0, ~6])
               diagonal-band tiles multiply by a mask tile (DVE)
               outT[j] += v_aug(t)^T attnT(t)       (PE, deferred one
                                                     pair so PE never
                                                     waits on exp)
            finalize: PE-transpose outT, divide by denominator column
            (DVE reciprocal + per-partition multiply), DMA out.

Matmul operands are bf16 (PE runs 1 cycle/row; fp32 is 4 and float32r is
SBUF-bandwidth-capped on this toolchain); accumulation is fp32 in PSUM.
"""
import os
import numpy as np
import ml_dtypes

import bass_rust
import concourse.bass as bass
import concourse.tile as tile
from concourse import mybir
from concourse.bass_utils import run_bass_kernel_spmd
from concourse.masks import make_identity

# ---------------------------------------------------------------- constants
P = 128          # partitions / sk tile
D = 512          # model dim
DK = 64          # key dim
S = 4096         # sequence
B = 4            # batch
CH = 512         # sq chunk width (one job)
NJ = 4           # jobs per core
KD = D // P      # k-tiles in the D contraction
NSK = S // P     # sk tiles
SQ = NJ * CH     # q rows per core
N_CORES = 8

F32 = mybir.dt.float32
BF16 = mybir.dt.bfloat16
F32R = mybir.dt.float32r

_DTMAP = {"bf16": BF16, "f32r": F32R, "f32": F32}
_NPMAP = {"bf16": ml_dtypes.bfloat16, "f32r": np.float32, "f32": np.float32}

_CFG = {
    "dt_proj": os.environ.get("K_DT_PROJ", "bf16"),
    "dt_sc": os.environ.get("K_DT_SC", "bf16"),
    "dt_pv": os.environ.get("K_DT_PV", "bf16"),
    "relu_dve": os.environ.get("K_RELU_DVE", "1") == "1",
    "mask_pool": os.environ.get("K_MASK_POOL", "0") == "1",
    "trace": os.environ.get("K_TRACE", "0") == "1",
}


# ------------------------------------------------- walrus codegen workarounds
def _patch_tile_drain():
    """This neuronxcc rejects >1 sync wait on a CTRL (Drain) instruction;
    TileContext's tail drain carries one wait per live semaphore.  Split the
    waits onto dedicated SP nops, one wait each."""
    from concourse.tile import TileContext

    if getattr(TileContext, "_drain_patched", False):
        return

    def _patched(self, tick_clock, wait_clock):
        nc = self.nc
        probe = nc.sync.nop(nofuse=True, hint="tail_wait_probe")
        wait_clock.add_sem_waits(
            probe.ins, bass_rust.ScopedClock({None: tick_clock.global_clock})
        )
        si = probe.ins.sync_info
        waits = list(si.on_wait) if si is not None else []
        probe.ins.sync_info = bass_rust.SyncInfo(on_wait=waits[:1], on_update=[])
        for w in waits[1:]:
            carrier = nc.sync.nop(nofuse=True, hint="tail_wait")
            carrier.ins.sync_info = bass_rust.SyncInfo(on_wait=[w], on_update=[])
        nc.sync.drain()

        nc.all_engine_barrier()
        assert self.sems is not None
        popped = nc._tile_sem_poison_stack.pop()
        assert popped is self._sem_poison
        nc.clear_and_free_semaphores(list(self.sems.allocated().values()))
        nc.all_engine_barrier()

    TileContext._drain_and_barrier = _patched
    TileContext._drain_patched = True


def _split_sync_waits(nc, max_waits: int = 1):
    """walrus here rejects >1 sync wait on at least CTRL and S3_LW (weight
    load) instruction structs.  Hoist excess waits onto same-engine NOPs
    placed immediately before the instruction (engine streams execute block
    order, so the waits still gate the instruction)."""
    counter = [0]
    for fn in nc.m.functions:
        for bb in fn.blocks:
            changed = False
            new = []
            for inst in bb.instructions:
                si = inst.sync_info
                waits = list(si.on_wait) if si is not None else []
                if len(waits) > max_waits:
                    changed = True
                    for w in waits[:-max_waits]:
                        counter[0] += 1
                        nop = bass_rust.InstNoOp(
                            name=f"I-waitsplit-{counter[0]}", engine=inst.engine
                        )
                        nop.bass_nofuse = True
                        nop.sync_info = bass_rust.SyncInfo(
                            on_wait=[w], on_update=[]
                        )
                        new.append(nop)
                    inst.sync_info = bass_rust.SyncInfo(
                        on_wait=waits[-max_waits:], on_update=list(si.on_update)
                    )
                new.append(inst)
            if changed:
                bb.instructions = new


# ---------------------------------------------------------------- program
def _build_program(causal: bool):
    _patch_tile_drain()
    nc = bass.Bass()

    DT_X = _DTMAP[_CFG["dt_proj"]]
    DT_QK = _DTMAP[_CFG["dt_sc"]]
    DT_AT = _DTMAP[_CFG["dt_pv"]]

    # chunk-contiguous host layouts: one DMA per 512-column chunk, each a
    # fully contiguous [128, KD*CH] block
    x1c = nc.declare_dram_parameter("x1c", [SQ // CH, P, KD * CH], DT_X,
                                    isOutput=False)
    x2c = nc.declare_dram_parameter("x2c", [S // CH, P, KD * CH], DT_X,
                                    isOutput=False)
    # packed projection weights: [Wq|Wq|Wk|Wk|Wv] (q/k duplicated so the
    # projection emits qT/kT replicated across both partition halves, which
    # lets the two K=64 score matmuls of a pair run concurrently on disjoint
    # PE row groups)
    WM = 5 * DK
    wall = nc.declare_dram_parameter("wall", [P, KD * WM], DT_X, isOutput=False)
    ball = nc.declare_dram_parameter("ball", [P, 3], F32, isOutput=False)
    masks = nc.declare_dram_parameter("masks", [8, P, CH], DT_AT, isOutput=False)
    ones = nc.declare_dram_parameter("ones", [P, NSK], DT_AT, isOutput=False)
    out = nc.declare_dram_parameter("out", [SQ, DK], F32, isOutput=True)

    E = [8 * j + 8 for j in range(NJ)] if causal else [NSK] * NJ

    Exp = mybir.ActivationFunctionType.Exp
    Relu = mybir.ActivationFunctionType.Relu

    def bias_relu(dst, src_psum, bias_sb):
        """dst = relu(src + bias), bias per-partition [p,1]."""
        if _CFG["relu_dve"]:
            nc.vector.tensor_scalar(
                dst, src_psum, bias_sb, 0.0,
                mybir.AluOpType.add, mybir.AluOpType.max,
            )
        else:
            nc.scalar.activation(out=dst, in_=src_psum, func=Relu,
                                 bias=bias_sb, scale=1.0)

    with tile.TileContext(nc) as tc:
        with (
            tc.tile_pool(name="const", bufs=1) as const,
            tc.tile_pool(name="xin", bufs=8) as xin,
            tc.tile_pool(name="resident", bufs=1) as res,
            tc.tile_pool(name="attn", bufs=6) as attn,
            tc.tile_pool(name="ostage", bufs=4) as ostage,
            tc.tile_pool(name="outps", bufs=2, space="PSUM") as outps,
            tc.tile_pool(name="pps", bufs=2, space="PSUM") as pps,
            tc.tile_pool(name="sps", bufs=2, space="PSUM") as sps,
        ):
            # ---------------- constants
            w_sb = const.tile([P, KD, WM], DT_X)
            b_sb = const.tile([P, 3], F32)
            ident = const.tile([P, P], F32)
            make_identity(nc, ident)
            identv = const.tile([P, P], DT_X)
            make_identity(nc, identv)

            qT_sb = res.tile([P, SQ], DT_QK)
            kT_sb = res.tile([P, S], DT_QK)
            vT_sb = res.tile([DK, S], DT_X)
            # inner stride 80 keeps each [*, st, 0:64] slice 32B-aligned for
            # the SBUF->SBUF DMA transpose that fills it
            VP = 80
            v_sb = res.tile([P, NSK, VP], DT_AT)
            nc.sync.dma_start(
                out=v_sb[:, :, DK:DK + 1],
                in_=ones.rearrange("p (n o) -> p n o", o=1),
            )

            # ---- all input DMAs issued up-front (SP executes triggers in
            # program order; interleaving them with output DMAs would stall
            # the input stream behind compute-dependent stores).  The first
            # x1/x2 chunks are split per-kd so the first projection matmuls
            # start as soon as their own 128KB slice lands.
            x1_first = [
                xin.tile([P, CH], DT_X, name=f"x1f{k}", tag="x1f")
                for k in range(KD)
            ]
            x2_first = [
                xin.tile([P, CH], DT_X, name=f"x2f{k}", tag="x2f")
                for k in range(KD)
            ]
            x1_tiles = [None] + [
                xin.tile([P, KD, CH], DT_X, name=f"x1t{c}", tag="x1c")
                for c in range(1, SQ // CH)
            ]
            x2_tiles = [None] + [
                xin.tile([P, KD, CH], DT_X, name=f"x2t{c}", tag="x2c")
                for c in range(1, S // CH)
            ]
            nc.sync.dma_start(out=w_sb, in_=wall.rearrange("p (kd m) -> p kd m", kd=KD))
            x1v0 = x1c[0].rearrange("p (kd s) -> p kd s", kd=KD)
            x2v0 = x2c[0].rearrange("p (kd s) -> p kd s", kd=KD)
            for k in range(KD):
                nc.sync.dma_start(out=x1_first[k], in_=x1v0[:, k, :])
            for k in range(KD):
                nc.sync.dma_start(out=x2_first[k], in_=x2v0[:, k, :])
            nc.sync.dma_start(out=b_sb, in_=ball[:, :])
            nc.sync.dma_start(
                out=x2_tiles[1],
                in_=x2c[1].rearrange("p (kd s) -> p kd s", kd=KD),
            )
            if causal:
                masks_sb = const.tile([P, 8, CH], DT_AT)
                nc.sync.dma_start(
                    out=masks_sb, in_=masks.rearrange("m p s -> p m s")
                )
            for ch in range(1, SQ // CH):
                nc.sync.dma_start(
                    out=x1_tiles[ch],
                    in_=x1c[ch].rearrange("p (kd s) -> p kd s", kd=KD),
                )
                for ch2 in (2 * ch, 2 * ch + 1):
                    nc.sync.dma_start(
                        out=x2_tiles[ch2],
                        in_=x2c[ch2].rearrange("p (kd s) -> p kd s", kd=KD),
                    )

            def proj_q_chunk(ch):
                pq = pps.tile([P, CH], F32, tag="pps")
                for kd in range(KD):
                    rhs = (x1_first[kd] if ch == 0
                           else x1_tiles[ch][:, kd, :])
                    nc.tensor.matmul(
                        pq, w_sb[:, kd, 0:P], rhs,
                        start=(kd == 0), stop=(kd == KD - 1),
                    )
                bias_relu(qT_sb[:, ch * CH:(ch + 1) * CH], pq, b_sb[:, 0:1])

            def proj_kv_chunk(ch):
                def rhs(kd):
                    return (x2_first[kd] if ch == 0
                            else x2_tiles[ch][:, kd, :])
                pk = pps.tile([P, CH], F32, tag="pps")
                for kd in range(KD):
                    nc.tensor.matmul(
                        pk, w_sb[:, kd, P:2 * P], rhs(kd),
                        start=(kd == 0), stop=(kd == KD - 1),
                    )
                bias_relu(kT_sb[:, ch * CH:(ch + 1) * CH], pk, b_sb[:, 1:2])
                pv = pps.tile([DK, CH], F32, tag="pps")
                for kd in range(KD):
                    nc.tensor.matmul(
                        pv, w_sb[:, kd, 2 * P:2 * P + DK], rhs(kd),
                        start=(kd == 0), stop=(kd == KD - 1),
                    )
                bias_relu(vT_sb[:, ch * CH:(ch + 1) * CH], pv, b_sb[0:DK, 2:3])

            def transpose_v(st):
                pt = pps.tile([P, DK], DT_X, tag="pps")
                nc.tensor.transpose(
                    pt, in_=vT_sb[:, st * P:(st + 1) * P],
                    identity=identv[:DK, :DK],
                )
                nc.vector.tensor_copy(v_sb[:, st, 0:DK], pt)

            def finalize_job(j, oT_ps):
                oT = ostage.tile([DK + 1, CH], F32, tag="oT")
                nc.vector.tensor_copy(oT, oT_ps)
                for blk in range(CH // P):
                    po = pps.tile([P, DK + 1], F32, tag="pps")
                    nc.tensor.transpose(
                        po,
                        in_=oT[:, blk * P:(blk + 1) * P],
                        identity=ident[:DK + 1, :DK + 1],
                    )
                    rec = ostage.tile([P, 1], F32, tag="rec")
                    nc.vector.reciprocal(rec, po[:, DK:DK + 1])
                    ot = ostage.tile([P, DK], F32, tag="ot")
                    nc.vector.tensor_scalar_mul(ot, po[:, 0:DK], rec)
                    r0 = j * CH + blk * P
                    nc.sync.dma_start(out=out[r0:r0 + P, :], in_=ot)

            def attention_job(j, new_tiles=(), finalize_prev=None):
                oT_ps = outps.tile([DK + 1, CH], F32, tag="outT")
                qslc = qT_sb[:, j * CH:(j + 1) * CH]
                npair = E[j] // 2
                DEPTH = 3        # PV trails the scores by 3 pairs so the PE
                pending = []     # stream never waits on a just-issued exp
                for pi in range(npair + DEPTH):
                    # spread the v transposes of this group's new key tiles
                    # across the early pairs (each tile is ready well before
                    # its PV consumes it)
                    for st in new_tiles[2 * pi:2 * pi + 2]:
                        transpose_v(st)
                    if pi == 1 and finalize_prev is not None:
                        finalize_prev()
                    if pi < npair:
                        sc = sps.tile([P, 1024], F32, tag="sc")
                        at = attn.tile([P, 1024], DT_AT, tag="attnT")
                        for half in range(2):
                            t = 2 * pi + half
                            lo = half * DK
                            nc.tensor.matmul(
                                sc[:, half * CH:(half + 1) * CH],
                                kT_sb[lo:lo + DK, t * P:(t + 1) * P],
                                qslc[lo:lo + DK, :],
                                start=True,
                                stop=True,
                            )
                        nc.scalar.activation(
                            out=at, in_=sc, func=Exp, scale=0.125
                        )
                        halves = []
                        for half in range(2):
                            t = 2 * pi + half
                            aslc = at[:, half * CH:(half + 1) * CH]
                            if causal and t >= E[j] - 8:
                                m = t - (E[j] - 8)
                                eng = nc.gpsimd if half == 1 else nc.vector
                                eng.tensor_tensor(
                                    aslc, aslc, masks_sb[:, m, :],
                                    mybir.AluOpType.mult,
                                )
                            halves.append((t, aslc))
                        pending.append(halves)
                    if pi >= DEPTH:
                        for t, aslc in pending.pop(0):
                            nc.tensor.matmul(
                                oT_ps,
                                v_sb[:, t, 0:DK + 1],
                                aslc,
                                start=(t == 0),
                                stop=(t == E[j] - 1),
                                skip_group_check=True,
                            )
                return lambda: finalize_job(j, oT_ps)

            # ---------------- interleaved emission: group j feeds job j
            fin = None
            for j in range(NJ):
                proj_q_chunk(j)
                lo, hi = 2 * j, 2 * j + 2
                if not causal:
                    lo, hi = (0, S // CH) if j == 0 else (0, 0)
                new_tiles = []
                for ch in range(lo, hi):
                    proj_kv_chunk(ch)
                    new_tiles.extend(
                        ch * (CH // P) + blk for blk in range(CH // P)
                    )
                if not causal and j == 0:
                    # all keys needed up-front: transpose before the job
                    for st in new_tiles:
                        transpose_v(st)
                    new_tiles = []
                fin = attention_job(j, new_tiles, finalize_prev=fin)
            fin()

    _split_sync_waits(nc)
    return nc


_PROGRAMS = {}


def _program(causal: bool):
    if causal not in _PROGRAMS:
        _PROGRAMS[causal] = _build_program(causal)
    return _PROGRAMS[causal]


def _host_masks(parity: int) -> np.ndarray:
    """masks[m] multiplies the exp'd [sk=128, sq=512] tile of the job whose
    diagonal band covers key tiles [E-8, E); m = position in that band."""
    sk = np.arange(P)[:, None]
    sq = np.arange(CH)[None, :]
    m = np.zeros((8, P, CH), np.float32)
    for i in range(8):
        if parity == 1:
            if i < 4:
                m[i] = 1.0
            else:
                r = i - 4
                m[i] = (sq >= r * P + sk).astype(np.float32)
        else:
            if i < 4:
                m[i] = (sq >= i * P + sk).astype(np.float32)
            else:
                m[i] = 0.0
    return m


def _chunked(xt_rows: np.ndarray, np_x) -> np.ndarray:
    """[rows, D] -> [nch, 128, KD*CH] where [ch, p, kd*CH+s] =
    x[ch*CH+s, kd*128+p]."""
    nch = xt_rows.shape[0] // CH
    a = xt_rows.reshape(nch, CH, KD, P).transpose(0, 3, 2, 1)
    return np.ascontiguousarray(a.reshape(nch, P, KD * CH).astype(np_x))


def kernel(x1, x2, Wq, bq, Wk, bk, Wv, bv, apply_mask):
    np_x = _NPMAP[_CFG["dt_proj"]]
    np_at = _NPMAP[_CFG["dt_pv"]]
    x1 = np.asarray(x1, dtype=np.float32)
    x2 = np.asarray(x2, dtype=np.float32)
    Wq_f = np.asarray(Wq, np.float32)
    Wk_f = np.asarray(Wk, np.float32)
    Wv_f = np.asarray(Wv, np.float32)
    # packed [Wq|Wq|Wk|Wk|Wv] rearranged to the SBUF chunk layout
    Wcat = np.concatenate([Wq_f, Wq_f, Wk_f, Wk_f, Wv_f], axis=1)  # [D, 320]
    WM = Wcat.shape[1]
    wall_h = np.ascontiguousarray(
        Wcat.reshape(KD, P, WM).transpose(1, 0, 2).reshape(P, KD * WM)
    ).astype(np_x)
    ball_h = np.zeros((P, 3), np.float32)
    ball_h[:, 0] = np.concatenate([bq, bq])
    ball_h[:, 1] = np.concatenate([bk, bk])
    ball_h[0:DK, 2] = bv
    causal = bool(int(np.asarray(apply_mask)))

    nc = _program(causal)

    x2c_h = [_chunked(x2[b], np_x) for b in range(B)]
    ones_h = np.ones((P, NSK), np_at)
    masks_h = [_host_masks(p).astype(np_at) for p in range(2)]

    in_maps = []
    for core in range(N_CORES):
        b, p = core // 2, core % 2
        xb = x1[b]                                   # [S, D]
        rows = np.concatenate(
            [xb[(2 * j + p) * CH:(2 * j + p + 1) * CH] for j in range(NJ)], axis=0
        )                                            # [2048, D]
        in_maps.append({
            "x1c": _chunked(rows, np_x),
            "x2c": x2c_h[b],
            "wall": wall_h, "ball": ball_h,
            "masks": masks_h[p],
            "ones": ones_h,
        })

    res = run_bass_kernel_spmd(
        nc, in_maps, core_ids=list(range(N_CORES)), trace=_CFG["trace"]
    )
    kernel.last_result = res

    outp = np.empty((B, S, DK), np.float32)
    for core in range(N_CORES):
        b, p = core // 2, core % 2
        o = res.results[core]["out"]                 # [2048, 64]
        for j in range(NJ):
            outp[b, (2 * j + p) * CH:(2 * j + p + 1) * CH] = \
                o[j * CH:(j + 1) * CH]
    return outp
